# revision 19
# baseline (speedup 1.0000x reference)
"""CIEDE-base color-difference loss kernel for 8 Trainium2 NeuronCores, v2.

Rebalanced vs v1: the ACT (scalar) engine was the bottleneck at ~26
element-ops/pixel (gamma ln+exp, cbrt ln+exp, sqrt ln+exp). v2 cuts ACT to
~14/px by replacing the exact sRGB gamma with a cubic vertex fit computed on
DVE + PE:

  lin(x) ~= A*(x+U)^2 + B3*(x+U)^3 + W      (max end-to-end rel err ~2e-4,
                                             validated incl. all bf16 steps)
  xu = x + U          -- DVE tensor_scalar, bf16 4x mode
  x2 = xu*xu          -- DVE tensor_tensor, bf16 2x mode
  x3 = x2*xu          -- DVE tensor_tensor, bf16 2x mode
  t  = (A*M)@x2 + (B3*M)@x3   -- PE bf16 matmuls into PSUM f32 (W folded into
                                 the Ln bias: rows of RGB2XYZ/white sum to 1)
  lt = Ln(t + W)      -- ACT from PSUM
  f  = Exp(lt/3)      -- ACT, one instr per x-tile (cbrt part 2), bf16 out
  dd = A2@f           -- PE: da/db for 3 column-chunks packed into 126
                         partitions via shifted weights
  sq = (bf16 copy of dd)^2    -- DVE
  s  = sum pairs      -- PE: 6 chunks packed into 126 partitions (2 shifted
                         weight variants), PSUM accumulate
  cd = Exp(0.5*Ln(s+eps), accum_out=acc)    -- ACT, per-pixel sqrt + free mean

Data is processed as one global column stream per core (4 image pairs x
12544 padded cols = 50176 = 8 x 6144 + 1024), so all tiles are full-size;
pair boundaries only matter at the final accumulation, where Exp instructions
are split into (partition-range x col-range) rectangles that each belong to
one image pair. The rectangle->pair mapping is emitted at build time (RECTS)
and applied host-side.

Layout: partition p = 21*c + k (c = channel slot: x1,y1,z1 then x2,y2,z2
after the matmul; r1,g1,b1,r2,g2,b2 before), k = pixel row 0..20. Each image
plane is host-padded to 21 rows x 12544 cols (pad 0.5 both images -> cd 0),
then the 4 pairs are laid side by side: DRAM tensor [2, 3, 21, 50176] f32.
Loads are single 3 MB SWDGE DMAs with f32->bf16 cast inline.
"""

import numpy as np
import ml_dtypes

B, C, H, W = 32, 3, 512, 512
HWPX = H * W                 # 262144 pixels per image
N_CORES = 8
B_LOC = B // N_CORES         # 4 image-pairs per core
ROWS = 21                    # partition-rows per image plane
ROWL = 12544                 # padded cols per plane (21*12544 = 263424)
PADPX = ROWS * ROWL
STREAM = B_LOC * ROWL        # 50176 global columns per core
XT = 6144                    # x-tile cols (12 chunks of 512)
N_XT = STREAM // XT          # 8 full x-tiles
RAG = STREAM - N_XT * XT     # 1024 ragged cols (2 chunks)
CHUNK = 512
N_CHUNK = STREAM // CHUNK    # 98 chunks

# cubic vertex gamma fit: lin ~= A_*(x+U_)^2 + B3_*(x+U_)^3 + W_
A_ = 0.533514513
B3_ = 0.398223045
U_ = 0.0343024908
W_ = -3.96809e-4

_RGB2XYZ = np.array([[0.4124564, 0.3575761, 0.1804375],
                     [0.2126729, 0.7151522, 0.0721750],
                     [0.0193339, 0.1191920, 0.9503041]], dtype=np.float64)
_WHITE = np.array([0.95047, 1.0, 1.08883], dtype=np.float64)

bf16 = ml_dtypes.bfloat16


def _build_weights():
    f32 = np.float32
    I21 = np.eye(ROWS, dtype=f32)
    Mp = (_RGB2XYZ / _WHITE[:, None])
    M6 = np.zeros((6, 6))
    M6[:3, :3] = Mp
    M6[3:, 3:] = Mp
    # gamma-stage: t[21c'+k] = sum_c M6[c',c] lin[21c+k], with the fit coeffs
    # a,b folded into the DVE-computed m = a + b*xu (lin = x2*m)
    WQ = np.kron(M6.T, I21).astype(bf16)                      # [126, 126]
    WC = np.kron((B3_ * M6).T, I21).astype(bf16)              # unused (legacy)
    # dd-stage: rows of AA = da, db coefficients over f slots
    AA = np.array([[500.0, -500.0, 0.0, -500.0, 500.0, 0.0],
                   [0.0, 200.0, -200.0, 0.0, -200.0, 200.0]], dtype=f32)
    base = np.kron(AA.T, I21).astype(f32)                     # [126, 42]
    W2all = np.zeros((126, 3 * 126), f32)
    for g in range(3):
        W2all[:, 126 * g + 42 * g: 126 * g + 42 * g + 42] = base
    # s-stage: sq126 partition 42g+21d+j (g chunk-offset, d da/db) -> s at
    # 21g+j (even sq tile -> rows 0..62) or 63+21g+j (odd -> 63..125)
    W4e = np.zeros((126, 126), f32)
    W4o = np.zeros((126, 126), f32)
    W4r = np.zeros((126, 126), f32)
    for g in range(3):
        for d in range(2):
            r0 = 42 * g + 21 * d
            W4e[r0:r0 + 21, 21 * g:21 * g + 21] = I21
            W4o[r0:r0 + 21, 63 + 21 * g:63 + 21 * g + 21] = I21
            if g < 2:
                W4r[r0:r0 + 21, 21 * g:21 * g + 21] = I21
    return (WQ, WC, W2all.astype(bf16), W4e.astype(bf16), W4o.astype(bf16),
            W4r.astype(bf16))


def _build_segs():
    """For each s-tile, the column segments needing separate accumulation.

    s-tile u covers chunks 6u..6u+5; partition block v (21 rows) holds chunk
    6u+v; chunk c covers stream cols [512c, 512c+512); pair p covers stream
    cols [12544p, 12544(p+1)). An Exp+accum instruction spans all 126
    partitions but must not mix pairs within one (partition, col) element, so
    tiles whose chunks straddle a pair boundary split at that column. The
    block->pair mapping per segment is applied host-side (accum_out sums are
    per-partition).
    """
    tiles = []
    for u in range(N_CHUNK // 6):                       # 16 full s-tiles
        splits = set()
        for v in range(6):
            c = 6 * u + v
            p_lo = (CHUNK * c) // ROWL
            p_hi = (CHUNK * c + CHUNK - 1) // ROWL
            if p_lo != p_hi:
                splits.add(ROWL * p_hi - CHUNK * c)
        pts = [0] + sorted(splits) + [CHUNK]
        tiles.append(list(zip(pts[:-1], pts[1:])))
    return tiles


_CACHE = {}


def _build_module(reps=1, variant="full", unroll=1):
    """variant: 'full' | 'dma' (loads + tiny DVE consumer only).

    unroll: python-unrolled copies of the stream pipeline per For_i
    iteration (bench use: amortizes the per-iteration all-engine barrier).
    """
    import concourse.bass as bass
    import concourse.bacc as bacc
    import concourse.tile as tile
    from concourse import mybir

    f32 = mybir.dt.float32
    bft = mybir.dt.bfloat16
    AF = mybir.ActivationFunctionType

    nc = bacc.Bacc(None, target_bir_lowering=False)

    imgsh = nc.dram_tensor("imgs", [2, C, ROWS, STREAM], f32,
                           kind="ExternalInput")
    wqh = nc.dram_tensor("wq", [126, 126], bft, kind="ExternalInput")
    wch = nc.dram_tensor("wc", [126, 126], bft, kind="ExternalInput")
    w2h = nc.dram_tensor("w2all", [126, 3 * 126], bft, kind="ExternalInput")
    w4eh = nc.dram_tensor("w4e", [126, 126], bft, kind="ExternalInput")
    w4oh = nc.dram_tensor("w4o", [126, 126], bft, kind="ExternalInput")
    w4rh = nc.dram_tensor("w4r", [126, 126], bft, kind="ExternalInput")

    s_tiles_segs = _build_segs()
    n_acc = sum(len(s) for s in s_tiles_segs) + 1       # +1 ragged tail
    outh = nc.dram_tensor("partials", [126, n_acc], f32, kind="ExternalOutput")

    def dram_src(col0, ncols):
        """[2 imgs x 3 ch x 21 rows, ncols] starting at stream col col0."""
        return bass.AP(tensor=imgsh, offset=col0,
                       ap=[[C * ROWS * STREAM, 2], [ROWS * STREAM, C],
                           [STREAM, ROWS], [1, ncols]])

    with tile.TileContext(nc) as tc:
        from contextlib import ExitStack
        with ExitStack() as ctx:
            singles = ctx.enter_context(tc.tile_pool(name="singles", bufs=1))
            xpool = ctx.enter_context(tc.tile_pool(name="x", bufs=3))
            xupool = ctx.enter_context(tc.tile_pool(name="xu", bufs=2))
            x2pool = ctx.enter_context(tc.tile_pool(name="x2", bufs=2))
            mpool = ctx.enter_context(tc.tile_pool(name="m", bufs=1))
            x3pool = ctx.enter_context(tc.tile_pool(name="x3", bufs=2))
            ltpool = ctx.enter_context(tc.tile_pool(name="lt", bufs=2))
            fpool = ctx.enter_context(tc.tile_pool(name="f", bufs=2))
            ddcpool = ctx.enter_context(tc.tile_pool(name="ddc", bufs=3))
            sqpool = ctx.enter_context(tc.tile_pool(name="sq", bufs=3))
            cdpool = ctx.enter_context(tc.tile_pool(name="cdln", bufs=3))
            qpool = ctx.enter_context(tc.tile_pool(name="q", bufs=2))
            # PSUM (8 banks): t [126,1024]x2 = 4, lt [126,1024] = 2,
            # dd [126,512] = 1, s [126,512] = 1. lt single-buffered is free:
            # ACT is serial, Ln_{j+1} follows Exp_j in program order anyway.
            tpool = ctx.enter_context(tc.tile_pool(name="t", bufs=2,
                                                   space="PSUM"))
            ltppool = ctx.enter_context(tc.tile_pool(name="ltp", bufs=1,
                                                     space="PSUM"))
            ddpool = ctx.enter_context(tc.tile_pool(name="dd", bufs=1,
                                                    space="PSUM"))
            spool = ctx.enter_context(tc.tile_pool(name="s", bufs=1,
                                                   space="PSUM"))

            wq = singles.tile([126, 126], bft)
            wc = singles.tile([126, 126], bft)
            w2a = singles.tile([126, 3 * 126], bft)
            w4e = singles.tile([126, 126], bft)
            w4o = singles.tile([126, 126], bft)
            w4r = singles.tile([126, 126], bft)
            nc.sync.dma_start(out=wq[:], in_=wqh[:, :])
            nc.sync.dma_start(out=wc[:], in_=wch[:, :])
            nc.sync.dma_start(out=w2a[:], in_=w2h[:, :])
            nc.sync.dma_start(out=w4e[:], in_=w4eh[:, :])
            nc.sync.dma_start(out=w4o[:], in_=w4oh[:, :])
            nc.sync.dma_start(out=w4r[:], in_=w4rh[:, :])

            acc = singles.tile([126, n_acc], f32)
            nc.vector.memset(acc[:], 0.0)

            wbias = singles.tile([128, 1], f32)
            nc.vector.memset(wbias[:], float(W_))
            ebias = singles.tile([128, 1], f32)
            nc.vector.memset(ebias[:], 1e-35)

            # dummy 1-elem Ln: walrus places the ~2.7us ACT table load
            # (natural_log_exp set) before the first ACTIVATE, so firing one
            # here overlaps the load with the first 3MB image DMA instead of
            # serializing it into the compute critical path
            warm = singles.tile([128, 1], f32)
            nc.scalar.activation(out=warm[0:1, 0:1], in_=ebias[0:1, 0:1],
                                 func=AF.Ln)

            if reps > 1:
                loop_cm = tc.For_i(0, reps, 1)
                loop_cm.__enter__()

            s_cur = None
            for _u in range(unroll):
              acc_col = 0
              for xt in range(N_XT + 1):
                ragged = xt == N_XT
                F = RAG if ragged else XT
                col0 = xt * XT

                # HWDGE f32 load at full rate (SWDGE cast-DMA measured ~2x
                # slow); the DVE shift below does the f32->bf16 conversion.
                x = xpool.tile([126, F], f32, tag="x")
                nc.sync.dma_start(out=x[:], in_=dram_src(col0, F))

                if variant == "dma":
                    red = qpool.tile([126, 1], f32, tag="red")
                    nc.vector.tensor_reduce(
                        out=red[:], in_=x[:, 0:CHUNK],
                        op=mybir.AluOpType.max, axis=mybir.AxisListType.X)
                    continue

                if variant == "act":
                    # pure-ACT mix mimicking full: 6x Ln(1024) + Exp(F) +
                    # one cd-like Ln/Exp(1024) pair per x-tile
                    lt = ltpool.tile([126, F], f32, tag="lt")
                    for j in range(F // 1024):
                        nc.scalar.activation(
                            out=lt[:, 1024 * j:1024 * j + 1024],
                            in_=x[:, 1024 * j:1024 * j + 1024],
                            func=AF.Ln, bias=wbias[0:126])
                    f = fpool.tile([126, F], bft, tag="f")
                    nc.scalar.activation(out=f[:], in_=lt[:], func=AF.Exp,
                                         scale=float(1.0 / 3.0))
                    cdln = cdpool.tile([126, 1024], f32, tag="cdln")
                    nc.scalar.activation(out=cdln[:], in_=lt[:, 0:1024],
                                         func=AF.Ln, bias=ebias[0:126])
                    nc.scalar.activation(out=cdln[:], in_=cdln[:],
                                         func=AF.Exp, scale=0.5,
                                         accum_out=acc[:, 0:1])
                    continue

                xu = xupool.tile([126, F], bft, tag="xu")
                nc.vector.tensor_scalar_add(xu[:], x[:], float(U_))
                x2 = x2pool.tile([126, F], bft, tag="x2")
                nc.vector.tensor_mul(x2[:], xu[:], xu[:])
                m = mpool.tile([126, F], bft, tag="m")
                nc.vector.tensor_scalar(m[:], xu[:], float(B3_), float(A_),
                                        op0=mybir.AluOpType.mult,
                                        op1=mybir.AluOpType.add)
                x3 = x3pool.tile([126, F], bft, tag="x3")   # lin = x2*m
                nc.vector.tensor_mul(x3[:], x2[:], m[:])

                if variant == "dve":
                    continue

                if variant == "pe":
                    # all matmuls with x2/x3 standing in for f/sq
                    for j in range(F // 1024):
                        tq = tpool.tile([126, 1024], f32, tag="t")
                        for h in range(2):
                            c0 = 1024 * j + 512 * h
                            nc.tensor.matmul(tq[:, 512 * h:512 * h + 512],
                                             wq[:], x2[:, c0:c0 + 512],
                                             start=True, stop=False)
                            nc.tensor.matmul(tq[:, 512 * h:512 * h + 512],
                                             wc[:], x3[:, c0:c0 + 512],
                                             start=False, stop=True)
                    n_grp = 4 if not ragged else 1
                    k_per = 3 if not ragged else 2
                    for g in range(n_grp):
                        dd = ddpool.tile([126, CHUNK], f32, tag="dd")
                        for k in range(k_per):
                            c0 = (k_per * g + k) * CHUNK
                            nc.tensor.matmul(dd[:],
                                             w2a[:, 126 * k:126 * k + 126],
                                             x2[:, c0:c0 + CHUNK],
                                             start=(k == 0),
                                             stop=(k == k_per - 1))
                        if not ragged:
                            G = 4 * xt + g
                            par = G % 2
                            if par == 0:
                                s_cur = spool.tile([126, CHUNK], f32, tag="s")
                            nc.tensor.matmul(s_cur[:],
                                             w4e[:] if par == 0 else w4o[:],
                                             x3[:, (k_per * g) * CHUNK:
                                                 (k_per * g) * CHUNK + CHUNK],
                                             start=(par == 0), stop=(par == 1))
                    continue

                # gamma matmul (lin = x3) + cbrt Ln/Exp per [126,1024] PSUM
                # tile; lt stays in PSUM so both ACT passes are PSUM-sourced
                # (the fast source class per the TRN2 SBUF-read errata)
                f = fpool.tile([126, F], bft, tag="f")
                for j in range(F // 1024):
                    tq = tpool.tile([126, 1024], f32, tag="t")
                    for h in range(2):
                        c0 = 1024 * j + 512 * h
                        nc.tensor.matmul(tq[:, 512 * h:512 * h + 512],
                                         wq[:], x3[:, c0:c0 + 512],
                                         start=True, stop=True)
                    ltp = ltppool.tile([126, 1024], f32, tag="ltp")
                    nc.scalar.activation(out=ltp[:], in_=tq[:], func=AF.Ln,
                                         bias=wbias[0:126])
                    nc.scalar.activation(out=f[:, 1024 * j:1024 * j + 1024],
                                         in_=ltp[:], func=AF.Exp,
                                         scale=float(1.0 / 3.0))

                # dd-stage: 3 chunks -> one [126,512] PSUM tile (2 if ragged)
                n_grp = 4 if not ragged else 1
                k_per = 3 if not ragged else 2
                for g in range(n_grp):
                    dd = ddpool.tile([126, CHUNK], f32, tag="dd")
                    for k in range(k_per):
                        c0 = (k_per * g + k) * CHUNK
                        nc.tensor.matmul(dd[:], w2a[:, 126 * k:126 * k + 126],
                                         f[:, c0:c0 + CHUNK],
                                         start=(k == 0), stop=(k == k_per - 1))
                    ddc = ddcpool.tile([126, CHUNK], bft, tag="ddc")
                    nc.vector.tensor_copy(ddc[:], dd[:])
                    sq = sqpool.tile([126, CHUNK], bft, tag="sq")
                    nc.vector.tensor_mul(sq[:], ddc[:], ddc[:])

                    if not ragged:
                        G = 4 * xt + g                  # global dd-group index
                        par = G % 2
                        if par == 0:
                            s_cur = spool.tile([126, CHUNK], f32, tag="s")
                        nc.tensor.matmul(s_cur[:], w4e[:] if par == 0 else w4o[:],
                                         sq[:], start=(par == 0),
                                         stop=(par == 1))
                        if par == 1:
                            u = G // 2
                            cdln = cdpool.tile([126, CHUNK], f32, tag="cdln")
                            nc.scalar.activation(out=cdln[:], in_=s_cur[:],
                                                 func=AF.Ln, bias=ebias[0:126])
                            for (c0, c1) in s_tiles_segs[u]:
                                nc.scalar.activation(
                                    out=cdln[:, c0:c1],
                                    in_=cdln[:, c0:c1], func=AF.Exp,
                                    scale=0.5,
                                    accum_out=acc[:, acc_col:acc_col + 1])
                                acc_col += 1
                    else:
                        sr = spool.tile([126, CHUNK], f32, tag="s")
                        nc.tensor.matmul(sr[:], w4r[:], sq[:],
                                         start=True, stop=True)
                        cdln = cdpool.tile([126, CHUNK], f32, tag="cdln")
                        nc.scalar.activation(out=cdln[0:42, :], in_=sr[0:42, :],
                                             func=AF.Ln, bias=ebias[0:42])
                        nc.scalar.activation(
                            out=cdln[0:42, :], in_=cdln[0:42, :],
                            func=AF.Exp, scale=0.5,
                            accum_out=acc[0:42, acc_col:acc_col + 1])
                        acc_col += 1

            if reps > 1:
                loop_cm.__exit__(None, None, None)

            if variant == "full":
                assert acc_col == n_acc, (acc_col, n_acc)
            nc.sync.dma_start(out=outh[:, :], in_=acc[:])

    nc.compile()
    return nc


def _acc_mapping():
    """Host-side: list over acc columns of [(p0, p1, pair), ...]."""
    s_tiles_segs = _build_segs()
    mapping = []
    for u, segs in enumerate(s_tiles_segs):
        for (c0, _c1) in segs:
            ents = []
            for v in range(6):
                pair = (CHUNK * (6 * u + v) + c0) // ROWL
                ents.append((21 * v, 21 * v + 21, pair))
            mapping.append(ents)
    mapping.append([(0, 42, B_LOC - 1)])                # ragged: chunks 96,97
    return mapping


def _get_module(reps=1):
    key = f"nc{reps}"
    if key not in _CACHE:
        _CACHE[key] = _build_module(reps)
    return _CACHE[key]


def make_in_maps(img1, img2):
    img1 = np.asarray(img1)
    img2 = np.asarray(img2)
    WQ, WC, W2all, W4e, W4o, W4r = _build_weights()
    in_maps = []
    for d in range(N_CORES):
        sl = slice(d * B_LOC, (d + 1) * B_LOC)
        m = {"wq": WQ, "wc": WC, "w2all": W2all, "w4e": W4e, "w4o": W4o,
             "w4r": W4r}
        imgs = np.full((2, C, ROWS, STREAM), 0.5, np.float32)
        for ii, img in enumerate((img1, img2)):
            pad = np.full((B_LOC, C, PADPX), 0.5, np.float32)
            pad[:, :, :HWPX] = img[sl].reshape(B_LOC, C, HWPX)
            # [B_LOC, C, ROWS, ROWL] -> [C, ROWS, B_LOC, ROWL]
            imgs[ii] = pad.reshape(B_LOC, C, ROWS, ROWL).transpose(
                1, 2, 0, 3).reshape(C, ROWS, STREAM)
        m["imgs"] = np.ascontiguousarray(imgs)
        in_maps.append(m)
    return in_maps


def kernel(img1, img2):
    import concourse.bass_utils as bass_utils

    img1 = np.ascontiguousarray(np.asarray(img1), dtype=np.float32)
    img2 = np.ascontiguousarray(np.asarray(img2), dtype=np.float32)
    assert img1.shape == (B, C, H, W)

    nc = _get_module()
    in_maps = make_in_maps(img1, img2)

    res = bass_utils.run_bass_kernel_spmd(nc, in_maps,
                                          core_ids=list(range(N_CORES)))
    _CACHE["last_results"] = res

    mapping = _acc_mapping()
    out = np.zeros(B, dtype=np.float64)
    for d in range(N_CORES):
        partials = res.results[d]["partials"].astype(np.float64)
        for col, ents in enumerate(mapping):
            for (p0, p1, pair) in ents:
                out[d * B_LOC + pair] += partials[p0:p1, col].sum()
    return (out / HWPX).astype(np.float32)


if __name__ == "__main__":
    i1 = np.load("/root/problem/img1.npy")
    i2 = np.load("/root/problem/img2.npy")
    print(kernel(i1, i2))
